# revision 33
# baseline (speedup 1.0000x reference)
"""Multi-head attention TRN2 Bass kernel (v3).

Problem: B=2, S=2048, E=1024, H=16, Dh=64; per-head QKV projection weights,
unmasked softmax(Q K^T / sqrt(Dh)) @ V, concat heads, out-projection.

Sharding: 8 cores = 2 batches x 4 head-groups (4 heads each). Each core
computes its batch/head-group's attention and a partial out-projection;
the host sums the 4 partials per batch and adds bo.

v2 (677us -> ~384us):
- x^T pre-transposed AND hi/lo bf16-split on the host; Q/K projections as
  3-term bf16 hi/lo (f32 PSUM); scores in 2 matmuls per j-tile with the
  softmax shift -m riding row 64 of the stationary (term1 =
  [Khi; ones; 0pad]^T [Qhi; -m; 0pad], term2 = [Khi; Klo]^T [Qlo; Qhi]);
  all score-path matmuls zero-padded to K=128 so fast-weight-load stays on;
  PSUM/SBUF ring tuning; phase 2 software-pipelined (next head's A
  matmul+reduce chunks interleaved into the current head's B/C/out-proj).

v3+ (384us -> ~367us):
- All device DMAs are contiguous per partition (host packs x chunks as
  [SBLK,128,ET,512] and weights as [128,...]): the descriptor-per-256B
  rearrange DMAs that delayed the first matmul to 20.8us are gone.
  Chunk DMAs are split 2-4 ways across queues (one queue streams only
  ~110 GB/s; a full 1MB chunk on one queue is slower than the compute
  it must hide under).
- ~28 identity warm-up matmuls run during the initial DMA wait so the
  HAM clock gate is at K=8/8 (2.4 GHz) when the first projection
  matmul lands.
- wv / first x_v chunk / deferred consts prefetch during the k phase:
  V-proj matmuls start right behind the k matmuls instead of 5us later.
  wo is pre-converted to bf16 on the host (no f32 bounce + DVE cast).
- qmA dropped: the A-pass reads qm directly. Row 64 of any qm column is
  0 (memset) until that column's own A step writes -m, and the km ones
  row contributes x*0 = 0, so the max is never contaminated. Saves
  16KB/partition of SBUF and a DMA per q-epilogue.
- the A-pass is driven from a single global job deque in heads order:
  the v-phase starts consuming it (DVE is idle there), and phase 2 keeps
  a running surplus so each head's -m rows land well before its B needs
  them (the old per-head generator issued the last -m DMA right at the
  B boundary, costing ~900ns per i-block).
- ps_proj is scoped to q/k and ps_v to the v block, freeing banks so
  ps_b can live through phase 1: B(h0, ib0) chunks (matmuls + exp) ride
  the v-phase where the ACT queue is otherwise idle.
- attT tiles share one 3-slot ring (2 slots re-couple the next head's
  exp to the C-pass reads; 2 tags x 2 bufs wastes 32KB/partition).

Failed experiments (reverted, kept here as a map of the wall): one exp
per [128,1024] 2-bank psb pair with pst folded into the ps_o ring -
serialized the C chain behind ACT copies, PE micro-idled, HAM
re-throttled, phase 2 ran 45us slower. Moving ctxT/out copies from ACT
to DVE - they queue behind 532ns MAX chains on the strict-FIFO DVE and
stall the C/O rings (+50us). The engine balance PE 332 / ACT 238 /
DVE 219 (us busy) with PE at 93-95% occupancy is the plateau; the
remaining gap to the ~330us PE floor is ~58 pipeline-restart matmuls
(+160ns each) at section boundaries plus startup/tail.
"""

import numpy as np

import concourse.bacc as bacc
import concourse.bass as bass
import concourse.mybir as mybir
import concourse.tile as tile
from concourse import masks
from concourse.bass_utils import run_bass_kernel_spmd

F32 = mybir.dt.float32
BF16 = mybir.dt.bfloat16
AX = mybir.AxisListType
AF = mybir.ActivationFunctionType
ALU = mybir.AluOpType

B, S, E, H, DH = 2, 2048, 1024, 16, 64
NCORES = 8
HPC = 4          # heads per core
NPAIR = 2        # head pairs per core
ET = E // 128    # 8 e-tiles
SBLK = 4         # 512-wide s blocks
IB = S // 512    # 4 i-blocks
JT = S // 128    # 16 j-tiles
JP = JT // 2     # j-tile pairs (one exp per pair)
MARGIN = 32.0    # safety margin for the hi-only approximate row max


def build_bass():
    nc = bacc.Bacc("TRN2", target_bir_lowering=False, debug=False,
                   num_devices=NCORES)
    xh_q = nc.dram_tensor("xh_q", [SBLK, 128, ET, 512], BF16, kind="ExternalInput")
    xl_q = nc.dram_tensor("xl_q", [SBLK, 128, ET, 512], BF16, kind="ExternalInput")
    xh_k = nc.dram_tensor("xh_k", [SBLK, 128, ET, 512], BF16, kind="ExternalInput")
    xl_k = nc.dram_tensor("xl_k", [SBLK, 128, ET, 512], BF16, kind="ExternalInput")
    xh_v = nc.dram_tensor("xh_v", [SBLK, 128, ET, 512], BF16, kind="ExternalInput")
    whq = nc.dram_tensor("whq", [128, NPAIR, ET, 128], BF16, kind="ExternalInput")
    wlq = nc.dram_tensor("wlq", [128, NPAIR, ET, 128], BF16, kind="ExternalInput")
    whk = nc.dram_tensor("whk", [128, NPAIR, ET, 128], BF16, kind="ExternalInput")
    wlk = nc.dram_tensor("wlk", [128, NPAIR, ET, 128], BF16, kind="ExternalInput")
    wv = nc.dram_tensor("wv", [128, ET, 256], BF16, kind="ExternalInput")
    bqs = nc.dram_tensor("bqs", [128, NPAIR], F32, kind="ExternalInput")
    bks = nc.dram_tensor("bks", [128, NPAIR], F32, kind="ExternalInput")
    bvb = nc.dram_tensor("bvb", [128, NPAIR, 128], F32, kind="ExternalInput")
    wo = nc.dram_tensor("wo", [128, NPAIR, E], BF16, kind="ExternalInput")
    out_p = nc.dram_tensor("out_p", [S, E], F32, kind="ExternalOutput")

    with tile.TileContext(nc) as tc:
        with (
            tc.tile_pool(name="const", bufs=1) as const_pool,
            tc.tile_pool(name="persist", bufs=1) as persist,
        ):
            ident_v = const_pool.tile([128, 128], BF16, name="ident_v")
            masks.make_identity(nc, ident_v[:])
            bqs_sb = const_pool.tile([128, NPAIR], F32, name="bqs")
            nc.sync.dma_start(bqs_sb[:], bqs[:])
            bks_sb = const_pool.tile([128, NPAIR], F32, name="bks")
            nc.sync.dma_start(bks_sb[:], bks[:])
            bvb_sb = const_pool.tile([128, HPC, 64], F32, name="bvb")
            bvb4 = bvb_sb
            wo_sb = const_pool.tile([128, NPAIR, E], BF16, name="wo")

            def load_deferred_consts():
                # issued mid-phase-1 so the startup DMA queues serve the
                # first x chunks / projection weights instead
                nc.sync.dma_start(bvb_sb[:], bvb[:])
                nc.sync.dma_start(wo_sb[:], wo[:])
            marg = const_pool.tile([128, 1], F32, name="marg")
            nc.gpsimd.memset(marg[:], -MARGIN)

            # per-head score operand tiles
            # qm: rows 0:64 = Q hi (scaled 1/8), row 64 = -(rowmax_hi+MARGIN)
            #     (0 until that column's A step runs; km's ones row hits it
            #     as x*0 = 0, so A can use qm directly as its stationary)
            # km: rows 0:64 = K hi, row 64 = ones
            # qlh: rows 0:64 = Q lo, rows 64:128 = Q hi
            # khl: rows 0:64 = K hi, rows 64:128 = K lo
            qm = [persist.tile([128, S], BF16, name=f"qm{h}") for h in range(HPC)]
            km = [persist.tile([128, S], BF16, name=f"km{h}") for h in range(HPC)]
            qlh = [persist.tile([128, S], BF16, name=f"qlh{h}") for h in range(HPC)]
            khl = [persist.tile([128, S], BF16, name=f"khl{h}") for h in range(HPC)]
            vhh = persist.tile([128, HPC, JT, 65], BF16, name="vhh")
            vh = [vhh[:, h] for h in range(HPC)]
            ctxT = [persist.tile([128, S], BF16, name=f"ctxT{c}") for c in range(2)]

            nc.gpsimd.memset(vhh[:, :, :, 64:65], 1.0)
            for h in range(HPC):
                nc.gpsimd.memset(km[h][64:128, :], 0.0)
                nc.gpsimd.memset(km[h][64:65, :], 1.0)
                nc.gpsimd.memset(qm[h][64:128, :], 0.0)

            # pools used by the A-pass (hoisted so A(0,*) can interleave
            # with the phase-1 v-projections)
            outer_small = tc.tile_pool(name="small", bufs=8)
            small = outer_small.__enter__()
            outer_psa = tc.tile_pool(name="ps_a", bufs=2, space="PSUM")
            ps_a = outer_psa.__enter__()

            # HAM warm-up: keep the PE busy during the initial DMA wait so
            # the clock gate is at 8/8 when the first real matmul lands.
            for _ in range(28):
                pw = ps_a.tile([128, 512], F32, name="psa")
                nc.tensor.matmul(pw[:, 0:128], ident_v[:], ident_v[:],
                                 start=True, stop=True)

            def gen_a(ib, h):
                # A: approximate row max (hi-only scores, [i,j]). Yields
                # after each matmul+reduce chunk; the -m row DMA is issued
                # per i-tile so its latency hides under remaining chunks.
                for it in range(4):
                    i0 = ib * 512 + it * 128
                    itsl = bass.ds(i0, 128)
                    nm44 = small.tile([128, 4], F32, name="nm44")
                    for jh in range(4):
                        jsl = bass.ts(jh, 512)
                        psa = ps_a.tile([128, 512], F32, name="psa")
                        nc.tensor.matmul(psa[:], qm[h][:, itsl],
                                         km[h][:, jsl],
                                         start=True, stop=True)
                        nc.vector.reduce_max(nm44[:, jh:jh + 1], psa[:],
                                             axis=AX.X)
                        yield
                    nm1 = small.tile([128, 1], BF16, name="nm1")
                    nc.vector.reduce_max(nm1[:], nm44[:],
                                         axis=AX.X, negate=True)
                    nc.sync.dma_start(qm[h][64:65, bass.ds(i0, 128)],
                                      nm1[:])
                    yield

            # global A-job pipeline in heads order: consumed opportunistically
            # from the v-phase onward, kept ahead of the B loop
            a_order = [(ib, p * 2 + hp) for ib in range(IB)
                       for p in range(NPAIR) for hp in range(2)]
            a_jobs = [[key, None] for key in a_order]
            a_pos = [0]

            def step_all(n):
                done = 0
                while done < n and a_pos[0] < len(a_jobs):
                    job = a_jobs[a_pos[0]]
                    if job[1] is None:
                        job[1] = gen_a(*job[0])
                    if next(job[1], "end") == "end":
                        a_pos[0] += 1
                    else:
                        done += 1

            def drain_until(key):
                idx = a_order.index(key)
                while a_pos[0] <= idx:
                    job = a_jobs[a_pos[0]]
                    if job[1] is None:
                        job[1] = gen_a(*job[0])
                    for _ in job[1]:
                        pass
                    a_pos[0] += 1

            # B machinery hoisted so B(h0, ib0) chunks (matmuls + exp) can
            # ride the v-phase, where the ACT queue is otherwise idle
            outer_psb = tc.tile_pool(name="ps_b", bufs=2, space="PSUM")
            ps_b = outer_psb.__enter__()
            outer_att = tc.tile_pool(name="attw", bufs=2)
            att_pool = outer_att.__enter__()
            attT = [None, None]

            def gen_b(ib, h, hp):
                # B: shifted scores + exp, [j, i] layout
                isl = bass.ts(ib, 512)
                attT[hp] = att_pool.tile([128, JT, 512], BF16,
                                         name="attT", bufs=3)
                for jt in range(JT):
                    psb = ps_b.tile([128, 512], F32, name="psb")
                    jsl = bass.ts(jt, 128)
                    nc.tensor.matmul(psb[:], km[h][:, jsl],
                                     qm[h][:, isl],
                                     start=True, stop=False)
                    nc.tensor.matmul(psb[:], khl[h][:, jsl],
                                     qlh[h][:, isl],
                                     start=False, stop=True)
                    nc.scalar.activation(attT[hp][:, jt, :],
                                         psb[:], AF.Exp, bias=marg[:, 0:1])
                    yield

            b0_hold = [None]

            def step_b0(n):
                # drive B(h0, ib0) once its A job (deque index 0) is done
                if b0_hold[0] == "done":
                    return
                if b0_hold[0] is None:
                    if a_pos[0] < 1:
                        return
                    b0_hold[0] = gen_b(0, 0, 0)
                for _ in range(n):
                    if next(b0_hold[0], "end") == "end":
                        b0_hold[0] = "done"
                        return

            # ---- phase 1: load + project ----
            with (
                tc.tile_pool(name="stage", bufs=2) as stage_pool,
                tc.tile_pool(name="vstage", bufs=1) as vstage_pool,
                tc.tile_pool(name="wght", bufs=2) as w_pool,
                tc.tile_pool(name="scr", bufs=3) as scr_pool,
            ):
                wv_hold = [None]
                xv0_hold = [None]
                # ps_proj scoped to q/k only so its banks free up for ps_v
                # while ps_b stays live (B rides the v-phase)
                ps_proj_cm = tc.tile_pool(name="ps_proj", bufs=3, space="PSUM")
                ps_proj = ps_proj_cm.__enter__()
                for which in ("q", "k"):
                    if which == "q":
                        xh_d, xl_d, wh_d, wl_d = xh_q, xl_q, whq, wlq
                        bias_sb, scl, qm_t, qlh_t = bqs_sb, 0.125, qm, qlh
                    else:
                        xh_d, xl_d, wh_d, wl_d = xh_k, xl_k, whk, wlk
                        bias_sb, scl, qm_t, qlh_t = bks_sb, 1.0, km, khl
                    if which == "q":
                        # startup path: split the weight DMAs per pair so
                        # they land on separate queues and finish sooner
                        wh_sb = w_pool.tile([128, NPAIR, ET, 128], BF16, name="wh_in")
                        wl_sb = w_pool.tile([128, NPAIR, ET, 128], BF16, name="wl_in")
                        for p_ in range(NPAIR):
                            nc.sync.dma_start(wh_sb[:, p_], wh_d[:, p_])
                            nc.sync.dma_start(wl_sb[:, p_], wl_d[:, p_])
                    else:
                        wh_sb = w_pool.tile([128, NPAIR, ET, 128], BF16, name="wh_in")
                        nc.sync.dma_start(wh_sb[:], wh_d[:])
                        wl_sb = w_pool.tile([128, NPAIR, ET, 128], BF16, name="wl_in")
                        nc.sync.dma_start(wl_sb[:], wl_d[:])
                    for sblk in range(SBLK):
                        xh_c = stage_pool.tile([128, ET, 512], BF16, name="xh_c")
                        xl_c = stage_pool.tile([128, ET, 512], BF16, name="xl_c")
                        nsp = 8 if (which == "q" and sblk == 0) else 2
                        for eg in range(nsp):
                            w_ = ET // nsp
                            esl = slice(eg * w_, (eg + 1) * w_)
                            nc.sync.dma_start(xh_c[:, esl],
                                              xh_d[sblk, :, esl])
                            nc.sync.dma_start(xl_c[:, esl],
                                              xl_d[sblk, :, esl])
                        if which == "k" and sblk == 1:
                            # prefetch the v-phase inputs while the k DMAs
                            # still have queue headroom, so V matmuls can
                            # start the moment the k matmuls finish
                            load_deferred_consts()
                            wv_hold[0] = w_pool.tile([128, ET, 256], BF16,
                                                     name="wv_in", bufs=1)
                            nc.sync.dma_start(wv_hold[0][:], wv[:])
                            xv0_hold[0] = vstage_pool.tile([128, ET, 512],
                                                           BF16, name="xv0")
                            nc.sync.dma_start(xv0_hold[0][:], xh_v[0])
                        if True:
                            for p in range(NPAIR):
                                psp = ps_proj.tile([128, 512], F32, name="psp")
                                for et in range(ET):
                                    nc.tensor.matmul(psp[:], wh_sb[:, p, et, :],
                                                     xh_c[:, et, :],
                                                     start=(et == 0), stop=False)
                                for et in range(ET):
                                    nc.tensor.matmul(psp[:], wl_sb[:, p, et, :],
                                                     xh_c[:, et, :],
                                                     start=False, stop=False)
                                for et in range(ET):
                                    nc.tensor.matmul(psp[:], wh_sb[:, p, et, :],
                                                     xl_c[:, et, :],
                                                     start=False,
                                                     stop=(et == ET - 1))
                                qex = scr_pool.tile([128, 512], F32, name="qex")
                                nc.scalar.activation(qex[:], psp[:], AF.Identity,
                                                     bias=bias_sb[:, p:p + 1],
                                                     scale=scl)
                                ssl = bass.ts(sblk, 512)
                                for hp in range(2):
                                    h = p * 2 + hp
                                    rsl = slice(hp * 64, hp * 64 + 64)
                                    if which == "q":
                                        # hi target: even -> qm[0:64];
                                        # odd -> qlh[64:128] (both direct)
                                        hi = qm_t[h][0:64, ssl] if hp == 0                                             else qlh_t[h][64:128, ssl]
                                        nc.scalar.activation(
                                            hi, psp[rsl, :], AF.Identity,
                                            bias=bias_sb[rsl, p:p + 1],
                                            scale=scl)
                                        if hp == 0:
                                            nc.vector.scalar_tensor_tensor(
                                                out=qlh_t[h][0:64, ssl],
                                                in0=qex[rsl, :],
                                                scalar=1.0, in1=hi,
                                                op0=ALU.mult, op1=ALU.subtract)
                                            nc.sync.dma_start(
                                                qlh_t[h][64:128, ssl], hi)
                                        else:
                                            ltmp = scr_pool.tile(
                                                [128, 512], BF16, name="ltmp")
                                            nc.vector.scalar_tensor_tensor(
                                                out=ltmp[64:128, :],
                                                in0=qex[rsl, :], scalar=1.0,
                                                in1=hi,
                                                op0=ALU.mult, op1=ALU.subtract)
                                            nc.sync.dma_start(
                                                qm_t[h][0:64, ssl], hi)
                                            nc.sync.dma_start(
                                                qlh_t[h][0:64, ssl],
                                                ltmp[64:128, :])
                                    else:
                                        # k: khl rows 0:64 = hi, 64:128 = lo
                                        if hp == 0:
                                            hi = qm_t[h][0:64, ssl]
                                            nc.scalar.activation(
                                                hi, psp[rsl, :], AF.Identity,
                                                bias=bias_sb[rsl, p:p + 1],
                                                scale=scl)
                                            nc.scalar.activation(
                                                qlh_t[h][0:64, ssl], psp[rsl, :],
                                                AF.Identity,
                                                bias=bias_sb[rsl, p:p + 1],
                                                scale=scl)
                                            ltmp = scr_pool.tile(
                                                [128, 512], BF16, name="ltmp")
                                            nc.vector.scalar_tensor_tensor(
                                                out=ltmp[0:64, :],
                                                in0=qex[rsl, :], scalar=1.0,
                                                in1=hi,
                                                op0=ALU.mult, op1=ALU.subtract)
                                            nc.sync.dma_start(
                                                qlh_t[h][64:128, ssl],
                                                ltmp[0:64, :])
                                        else:
                                            hi = scr_pool.tile(
                                                [128, 512], BF16, name="ktmp")
                                            nc.scalar.activation(
                                                hi[64:128, :], psp[rsl, :],
                                                AF.Identity,
                                                bias=bias_sb[rsl, p:p + 1],
                                                scale=scl)
                                            nc.sync.dma_start(
                                                qm_t[h][0:64, ssl],
                                                hi[64:128, :])
                                            nc.sync.dma_start(
                                                qlh_t[h][0:64, ssl],
                                                hi[64:128, :])
                                            nc.vector.scalar_tensor_tensor(
                                                out=qlh_t[h][64:128, ssl],
                                                in0=qex[rsl, :], scalar=1.0,
                                                in1=hi[64:128, :],
                                                op0=ALU.mult, op1=ALU.subtract)
                ps_proj_cm.__exit__(None, None, None)

                # ---- v projection, interleaved with A jobs + B(h0, ib0) ----
                ps_v_cm = tc.tile_pool(name="ps_v", bufs=3, space="PSUM")
                ps_v = ps_v_cm.__enter__()
                wv_sb = wv_hold[0]
                for sblk in range(SBLK):
                    if sblk == 0:
                        xh_c = xv0_hold[0]
                    else:
                        xh_c = stage_pool.tile([128, ET, 512], BF16, name="xh_c")
                        for eg in range(2):
                            esl = slice(eg * 4, eg * 4 + 4)
                            nc.sync.dma_start(xh_c[:, esl],
                                              xh_v[sblk, :, esl])
                    # V: out [s, d(4 heads)] per 128-s tile, N=256
                    for st2 in range(4):
                        st = sblk * 4 + st2
                        s2 = bass.ts(st2, 128)
                        psv = ps_v.tile([128, 4, 64], F32, name="psv")
                        for et in range(ET):
                            nc.tensor.matmul(psv[:],
                                             xh_c[:, et, s2],
                                             wv_sb[:, et, :],
                                             start=(et == 0),
                                             stop=(et == ET - 1))
                        nc.vector.scalar_tensor_tensor(
                            out=vhh[:, :, st, 0:64],
                            in0=psv[:], scalar=1.0,
                            in1=bvb4[:],
                            op0=ALU.mult, op1=ALU.add)
                        step_all(3)
                ps_v_cm.__exit__(None, None, None)

            # ---- phase 2: attention + out-projection ----
            with (
                tc.tile_pool(name="ctxn", bufs=6) as ctx_pool,
                tc.tile_pool(name="outs", bufs=3) as out_pool,
                tc.tile_pool(name="ps_s", bufs=2, space="PSUM") as ps_s,
                tc.tile_pool(name="ps_o", bufs=2, space="PSUM") as ps_o,
            ):
                def gen_c(ib, p):
                    # C: att @ V-hat, normalize, transpose ctx. The psc
                    # accumulator and pso share the ps_o bank ring. For the
                    # second pair the out-proj for each i-tile is emitted
                    # right after its ctxT rows land: no separate O section
                    # means fewer PE pipeline restarts, and the out copies
                    # drain on ACT during C instead of delaying the next
                    # B section's exp stream.
                    for it in range(4):
                        i0 = ib * 512 + it * 128
                        ctxn = ctx_pool.tile([128, 128], BF16, name="ctxn")
                        for hp in range(2):
                            h = p * 2 + hp
                            psc = ps_o.tile([128, 512], F32, name="pso")
                            for jt in range(JT):
                                nc.tensor.matmul(
                                    psc[:, 0:65],
                                    attT[hp][:, jt, bass.ts(it, 128)],
                                    vh[h][:, jt, :],
                                    start=(jt == 0), stop=(jt == JT - 1))
                            recip = small.tile([128, 1], F32, name="recip")
                            nc.vector.reciprocal(recip[:], psc[:, 64:65])
                            nc.vector.tensor_scalar_mul(
                                ctxn[:, bass.ds(hp * 64, 64)],
                                psc[:, 0:64], recip[:])
                            yield
                        pst = ps_s.tile([128, 128], BF16, name="pst")
                        nc.tensor.transpose(pst[:], ctxn[:], ident_v[:])
                        nc.scalar.copy(ctxT[p][:, bass.ds(i0, 128)], pst[:])
                        if p == 1:
                            for eh in range(2):
                                pso = ps_o.tile([128, 512], F32, name="pso")
                                for ct in range(2):
                                    nc.tensor.matmul(
                                        pso[:],
                                        ctxT[ct][:, bass.ds(i0, 128)],
                                        wo_sb[:, ct, bass.ts(eh, 512)],
                                        start=(ct == 0), stop=(ct == 1))
                                outsb = out_pool.tile([128, 512], F32,
                                                      name="outsb")
                                if ib == IB - 1 and it == 3 and eh == 1:
                                    # final chunk: DVE copy overlaps the
                                    # previous chunk's ACT copy at the tail
                                    nc.vector.tensor_copy(outsb[:], pso[:])
                                else:
                                    nc.scalar.copy(outsb[:], pso[:])
                                nc.sync.dma_start(
                                    out_p[bass.ds(i0, 128),
                                          bass.ts(eh, 512)], outsb[:])
                                yield

                # software pipeline over (ib, head): upcoming heads' A work
                # (vector-paced, from the global deque) is fed between every
                # PE-dense chunk of the current head's B, C and out-proj, so
                # the PE queue never sits on a bare reduce-wait chain and
                # each head's -m rows land ahead of its B.
                heads = [(ib, p, hp) for ib in range(IB)
                         for p in range(NPAIR) for hp in range(2)]
                for ib, p, hp in heads:
                    h = p * 2 + hp
                    drain_until((ib, h))
                    if ib == 0 and h == 0:
                        # B(h0, ib0) may be partially (or fully) consumed
                        # by the v-phase interleave; finish the remainder
                        step_b0(0)
                        if b0_hold[0] != "done":
                            for _ in b0_hold[0]:
                                step_all(1)
                            b0_hold[0] = "done"
                    else:
                        for _ in gen_b(ib, h, hp):
                            step_all(1)
                    if hp == 1:
                        for _ in gen_c(ib, p):
                            step_all(1)
            outer_att.__exit__(None, None, None)
            outer_psb.__exit__(None, None, None)
            outer_psa.__exit__(None, None, None)
            outer_small.__exit__(None, None, None)
    nc.finalize()
    return nc


_NC_CACHE = None


def _get_nc():
    global _NC_CACHE
    if _NC_CACHE is None:
        _NC_CACHE = build_bass()
    return _NC_CACHE


def _prep_core_inputs(inputs, core):
    bf16 = mybir.dt.np(BF16)
    b, hg = core // 4, core % 4
    h0 = hg * HPC
    q, k, v = inputs["q"], inputs["k"], inputs["v"]
    Wq, Wk, Wv = inputs["Wq"], inputs["Wk"], inputs["Wv"]
    bq, bk, bv = inputs["bq"], inputs["bk"], inputs["bv"]
    Wo = inputs["Wo"]

    def split_hl(x):
        xh = x.astype(bf16)
        xl = (x - xh.astype(np.float32)).astype(bf16)
        return xh, xl

    def xt_tiles(x):
        # [S, E] -> [ET, 128, S]
        return np.ascontiguousarray(x.T).reshape(ET, 128, S)

    def chunk_pack(xt):
        # [ET, 128, S] -> [SBLK, 128, ET, 512]: per-partition contiguous
        # 8KB runs per chunk DMA
        return np.ascontiguousarray(
            xt.reshape(ET, 128, SBLK, 512).transpose(2, 1, 0, 3))

    def pack_w(W):
        # [128e, NPAIR, ET, 128]: pair p, e-tile t -> [W[h0+2p] | W[h0+2p+1]]
        out = np.empty((NPAIR, ET, 128, 128), np.float32)
        for p in range(NPAIR):
            pair = np.concatenate([W[h0 + 2 * p], W[h0 + 2 * p + 1]], axis=1)
            out[p] = pair.reshape(ET, 128, 128)
        return np.ascontiguousarray(out.transpose(2, 0, 1, 3))

    def pack_bcol(bias, scale):
        out = np.empty((128, NPAIR), np.float32)
        for p in range(NPAIR):
            out[:, p] = np.concatenate(
                [bias[h0 + 2 * p], bias[h0 + 2 * p + 1]]) * scale
        return out

    xh_q, xl_q = split_hl(xt_tiles(q[b]))
    xh_k, xl_k = split_hl(xt_tiles(k[b]))
    xh_v = xt_tiles(v[b]).astype(bf16)
    whq, wlq = split_hl(pack_w(Wq))
    whk, wlk = split_hl(pack_w(Wk))
    wv_c = np.ascontiguousarray(
        np.concatenate([Wv[h0 + j] for j in range(HPC)],
                       axis=1).reshape(ET, 128, 256).transpose(1, 0, 2)
    ).astype(bf16)

    bvb = np.empty((128, NPAIR, 128), np.float32)
    for p in range(NPAIR):
        bvb[:, p, :] = np.concatenate([bv[h0 + 2 * p], bv[h0 + 2 * p + 1]])[None, :]

    wo_rows = Wo[h0 * DH:(h0 + HPC) * DH, :]  # [256, E]
    wo_c = np.ascontiguousarray(
        wo_rows.reshape(NPAIR, 128, E).transpose(1, 0, 2)).astype(bf16)
    return {
        "xh_q": chunk_pack(xh_q), "xl_q": chunk_pack(xl_q),
        "xh_k": chunk_pack(xh_k), "xl_k": chunk_pack(xl_k),
        "xh_v": chunk_pack(xh_v),
        "whq": whq, "wlq": wlq, "whk": whk, "wlk": wlk, "wv": wv_c,
        "bqs": pack_bcol(bq, 0.125), "bks": pack_bcol(bk, 1.0), "bvb": bvb,
        "wo": wo_c,
    }


def run(inputs, trace=False, **kw):
    inputs = {k: np.asarray(v) for k, v in inputs.items()}
    nc = _get_nc()
    in_maps = [_prep_core_inputs(inputs, c) for c in range(NCORES)]
    res = run_bass_kernel_spmd(nc, in_maps, list(range(NCORES)), trace=trace, **kw)
    bo = inputs["bo"]
    out = np.empty((B, S, E), np.float32)
    for b in range(B):
        acc = res.results[b * 4]["out_p"].astype(np.float32)
        for c in range(b * 4 + 1, b * 4 + 4):
            acc = acc + res.results[c]["out_p"]
        out[b] = acc + bo[None, :]
    return out, res


def kernel(**inputs):
    out, _ = run(inputs)
    return out


# revision 34
# speedup vs baseline: 1.0118x; 1.0118x over previous
"""Multi-head attention TRN2 Bass kernel (v3).

Problem: B=2, S=2048, E=1024, H=16, Dh=64; per-head QKV projection weights,
unmasked softmax(Q K^T / sqrt(Dh)) @ V, concat heads, out-projection.

Sharding: 8 cores = 2 batches x 4 head-groups (4 heads each). Each core
computes its batch/head-group's attention and a partial out-projection;
the host sums the 4 partials per batch and adds bo.

v2 (677us -> ~384us):
- x^T pre-transposed AND hi/lo bf16-split on the host; Q/K projections as
  3-term bf16 hi/lo (f32 PSUM); scores in 2 matmuls per j-tile with the
  softmax shift -m riding row 64 of the stationary (term1 =
  [Khi; ones; 0pad]^T [Qhi; -m; 0pad], term2 = [Khi; Klo]^T [Qlo; Qhi]);
  all score-path matmuls zero-padded to K=128 so fast-weight-load stays on;
  PSUM/SBUF ring tuning; phase 2 software-pipelined (next head's A
  matmul+reduce chunks interleaved into the current head's B/C/out-proj).

v3+ (384us -> ~367us):
- All device DMAs are contiguous per partition (host packs x chunks as
  [SBLK,128,ET,512] and weights as [128,...]): the descriptor-per-256B
  rearrange DMAs that delayed the first matmul to 20.8us are gone.
  Chunk DMAs are split 2-4 ways across queues (one queue streams only
  ~110 GB/s; a full 1MB chunk on one queue is slower than the compute
  it must hide under).
- ~28 identity warm-up matmuls run during the initial DMA wait so the
  HAM clock gate is at K=8/8 (2.4 GHz) when the first projection
  matmul lands.
- wv / first x_v chunk / deferred consts prefetch during the k phase:
  V-proj matmuls start right behind the k matmuls instead of 5us later.
  wo is pre-converted to bf16 on the host (no f32 bounce + DVE cast).
- qmA dropped: the A-pass reads qm directly. Row 64 of any qm column is
  0 (memset) until that column's own A step writes -m, and the km ones
  row contributes x*0 = 0, so the max is never contaminated. Saves
  16KB/partition of SBUF and a DMA per q-epilogue.
- the A-pass is driven from a single global job deque in heads order:
  the v-phase starts consuming it (DVE is idle there), and phase 2 keeps
  a running surplus so each head's -m rows land well before its B needs
  them (the old per-head generator issued the last -m DMA right at the
  B boundary, costing ~900ns per i-block).
- ps_proj is scoped to q/k and ps_v to the v block, freeing banks so
  ps_b can live through phase 1: B(h0, ib0) chunks (matmuls + exp) ride
  the v-phase where the ACT queue is otherwise idle.
- attT tiles share one 3-slot ring (2 slots re-couple the next head's
  exp to the C-pass reads; 2 tags x 2 bufs wastes 32KB/partition).

Failed experiments (reverted, kept here as a map of the wall): one exp
per [128,1024] 2-bank psb pair with pst folded into the ps_o ring -
serialized the C chain behind ACT copies, PE micro-idled, HAM
re-throttled, phase 2 ran 45us slower. Moving ctxT/out copies from ACT
to DVE - they queue behind 532ns MAX chains on the strict-FIFO DVE and
stall the C/O rings (+50us). The engine balance PE 332 / ACT 238 /
DVE 219 (us busy) with PE at 93-95% occupancy is the plateau; the
remaining gap to the ~330us PE floor is ~58 pipeline-restart matmuls
(+160ns each) at section boundaries plus startup/tail.
"""

import numpy as np

import concourse.bacc as bacc
import concourse.bass as bass
import concourse.mybir as mybir
import concourse.tile as tile
from concourse import masks
from concourse.bass_utils import run_bass_kernel_spmd

F32 = mybir.dt.float32
BF16 = mybir.dt.bfloat16
AX = mybir.AxisListType
AF = mybir.ActivationFunctionType
ALU = mybir.AluOpType

B, S, E, H, DH = 2, 2048, 1024, 16, 64
NCORES = 8
HPC = 4          # heads per core
NPAIR = 2        # head pairs per core
ET = E // 128    # 8 e-tiles
SBLK = 4         # 512-wide s blocks
IB = S // 512    # 4 i-blocks
JT = S // 128    # 16 j-tiles
JP = JT // 2     # j-tile pairs (one exp per pair)
MARGIN = 32.0    # safety margin for the hi-only approximate row max


def build_bass():
    nc = bacc.Bacc("TRN2", target_bir_lowering=False, debug=False,
                   num_devices=NCORES)
    xh_q = nc.dram_tensor("xh_q", [SBLK, 128, ET, 512], BF16, kind="ExternalInput")
    xl_q = nc.dram_tensor("xl_q", [SBLK, 128, ET, 512], BF16, kind="ExternalInput")
    xh_k = nc.dram_tensor("xh_k", [SBLK, 128, ET, 512], BF16, kind="ExternalInput")
    xl_k = nc.dram_tensor("xl_k", [SBLK, 128, ET, 512], BF16, kind="ExternalInput")
    xh_v = nc.dram_tensor("xh_v", [SBLK, 128, ET, 512], BF16, kind="ExternalInput")
    whq = nc.dram_tensor("whq", [128, NPAIR, ET, 128], BF16, kind="ExternalInput")
    wlq = nc.dram_tensor("wlq", [128, NPAIR, ET, 128], BF16, kind="ExternalInput")
    whk = nc.dram_tensor("whk", [128, NPAIR, ET, 128], BF16, kind="ExternalInput")
    wlk = nc.dram_tensor("wlk", [128, NPAIR, ET, 128], BF16, kind="ExternalInput")
    wv = nc.dram_tensor("wv", [128, ET, 256], BF16, kind="ExternalInput")
    bqs = nc.dram_tensor("bqs", [128, NPAIR], F32, kind="ExternalInput")
    bks = nc.dram_tensor("bks", [128, NPAIR], F32, kind="ExternalInput")
    bvb = nc.dram_tensor("bvb", [128, NPAIR, 128], F32, kind="ExternalInput")
    wo = nc.dram_tensor("wo", [128, NPAIR, E], BF16, kind="ExternalInput")
    out_p = nc.dram_tensor("out_p", [S, E], F32, kind="ExternalOutput")

    with tile.TileContext(nc) as tc:
        with (
            tc.tile_pool(name="const", bufs=1) as const_pool,
            tc.tile_pool(name="persist", bufs=1) as persist,
        ):
            ident_v = const_pool.tile([128, 128], BF16, name="ident_v")
            masks.make_identity(nc, ident_v[:])
            bqs_sb = const_pool.tile([128, NPAIR], F32, name="bqs")
            nc.sync.dma_start(bqs_sb[:], bqs[:])
            bks_sb = const_pool.tile([128, NPAIR], F32, name="bks")
            nc.sync.dma_start(bks_sb[:], bks[:])
            bvb_sb = const_pool.tile([128, HPC, 64], F32, name="bvb")
            bvb4 = bvb_sb
            wo_sb = const_pool.tile([128, NPAIR, E], BF16, name="wo")

            def load_deferred_consts():
                # issued mid-phase-1 so the startup DMA queues serve the
                # first x chunks / projection weights instead
                nc.sync.dma_start(bvb_sb[:], bvb[:])
                nc.sync.dma_start(wo_sb[:], wo[:])
            marg = const_pool.tile([128, 1], F32, name="marg")
            nc.gpsimd.memset(marg[:], -MARGIN)

            # per-head score operand tiles
            # qm: rows 0:64 = Q hi (scaled 1/8), row 64 = -(rowmax_hi+MARGIN)
            #     (0 until that column's A step runs; km's ones row hits it
            #     as x*0 = 0, so A can use qm directly as its stationary)
            # km: rows 0:64 = K hi, row 64 = ones
            # qlh: rows 0:64 = Q lo, rows 64:128 = Q hi
            # khl: rows 0:64 = K hi, rows 64:128 = K lo
            qm = [persist.tile([128, S], BF16, name=f"qm{h}") for h in range(HPC)]
            km = [persist.tile([128, S], BF16, name=f"km{h}") for h in range(HPC)]
            qlh = [persist.tile([128, S], BF16, name=f"qlh{h}") for h in range(HPC)]
            khl = [persist.tile([128, S], BF16, name=f"khl{h}") for h in range(HPC)]
            vhh = persist.tile([128, HPC, JT, 65], BF16, name="vhh")
            vh = [vhh[:, h] for h in range(HPC)]
            ctxT = [persist.tile([128, S], BF16, name=f"ctxT{c}") for c in range(2)]

            nc.gpsimd.memset(vhh[:, :, :, 64:65], 1.0)
            for h in range(HPC):
                nc.gpsimd.memset(km[h][64:128, :], 0.0)
                nc.gpsimd.memset(km[h][64:65, :], 1.0)
                nc.gpsimd.memset(qm[h][64:128, :], 0.0)

            # pools used by the A-pass (hoisted so A(0,*) can interleave
            # with the phase-1 v-projections)
            outer_small = tc.tile_pool(name="small", bufs=8)
            small = outer_small.__enter__()
            outer_psa = tc.tile_pool(name="ps_a", bufs=2, space="PSUM")
            ps_a = outer_psa.__enter__()

            # HAM warm-up: keep the PE busy during the initial DMA wait so
            # the clock gate is at 8/8 when the first real matmul lands.
            for _ in range(28):
                pw = ps_a.tile([128, 512], F32, name="psa")
                nc.tensor.matmul(pw[:, 0:128], ident_v[:], ident_v[:],
                                 start=True, stop=True)

            def gen_a(ib, h):
                # A: approximate row max (hi-only scores, [i,j]). Yields
                # after each matmul+reduce chunk; the -m row DMA is issued
                # per i-tile so its latency hides under remaining chunks.
                for it in range(4):
                    i0 = ib * 512 + it * 128
                    itsl = bass.ds(i0, 128)
                    nm44 = small.tile([128, 4], F32, name="nm44")
                    for jh in range(4):
                        jsl = bass.ts(jh, 512)
                        psa = ps_a.tile([128, 512], F32, name="psa")
                        nc.tensor.matmul(psa[:], qm[h][:, itsl],
                                         km[h][:, jsl],
                                         start=True, stop=True)
                        nc.vector.reduce_max(nm44[:, jh:jh + 1], psa[:],
                                             axis=AX.X)
                        yield
                    nm1 = small.tile([128, 1], BF16, name="nm1")
                    nc.vector.reduce_max(nm1[:], nm44[:],
                                         axis=AX.X, negate=True)
                    nc.sync.dma_start(qm[h][64:65, bass.ds(i0, 128)],
                                      nm1[:])
                    yield

            # global A-job pipeline in heads order: consumed opportunistically
            # from the v-phase onward, kept ahead of the B loop
            a_order = [(ib, p * 2 + hp) for ib in range(IB)
                       for p in range(NPAIR) for hp in range(2)]
            a_jobs = [[key, None] for key in a_order]
            a_pos = [0]

            def step_all(n):
                done = 0
                while done < n and a_pos[0] < len(a_jobs):
                    job = a_jobs[a_pos[0]]
                    if job[1] is None:
                        job[1] = gen_a(*job[0])
                    if next(job[1], "end") == "end":
                        a_pos[0] += 1
                    else:
                        done += 1

            def drain_until(key):
                idx = a_order.index(key)
                while a_pos[0] <= idx:
                    job = a_jobs[a_pos[0]]
                    if job[1] is None:
                        job[1] = gen_a(*job[0])
                    for _ in job[1]:
                        pass
                    a_pos[0] += 1

            # B machinery hoisted so B(h0, ib0) chunks (matmuls + exp) can
            # ride the v-phase, where the ACT queue is otherwise idle
            outer_psb = tc.tile_pool(name="ps_b", bufs=2, space="PSUM")
            ps_b = outer_psb.__enter__()
            outer_att = tc.tile_pool(name="attw", bufs=2)
            att_pool = outer_att.__enter__()
            attT = [None, None]

            def gen_b(ib, h, hp):
                # B: shifted scores + exp, [j, i] layout
                isl = bass.ts(ib, 512)
                attT[hp] = att_pool.tile([128, JT, 512], BF16,
                                         name="attT", bufs=3)
                for jt in range(JT):
                    psb = ps_b.tile([128, 512], F32, name="psb")
                    jsl = bass.ts(jt, 128)
                    nc.tensor.matmul(psb[:], km[h][:, jsl],
                                     qm[h][:, isl],
                                     start=True, stop=False)
                    nc.tensor.matmul(psb[:], khl[h][:, jsl],
                                     qlh[h][:, isl],
                                     start=False, stop=True)
                    nc.scalar.activation(attT[hp][:, jt, :],
                                         psb[:], AF.Exp, bias=marg[:, 0:1])
                    yield

            b0_hold = [None]

            def step_b0(n):
                # drive B(h0, ib0) once its A job (deque index 0) is done
                if b0_hold[0] == "done":
                    return
                if b0_hold[0] is None:
                    if a_pos[0] < 1:
                        return
                    b0_hold[0] = gen_b(0, 0, 0)
                for _ in range(n):
                    if next(b0_hold[0], "end") == "end":
                        b0_hold[0] = "done"
                        return

            # ---- phase 1: load + project ----
            with (
                tc.tile_pool(name="stage", bufs=2) as stage_pool,
                tc.tile_pool(name="vstage", bufs=1) as vstage_pool,
                tc.tile_pool(name="wght", bufs=2) as w_pool,
                tc.tile_pool(name="scr", bufs=3) as scr_pool,
            ):
                wv_hold = [None]
                xv0_hold = [None]
                # ps_proj scoped to q/k only so its banks free up for ps_v
                # while ps_b stays live (B rides the v-phase)
                ps_proj_cm = tc.tile_pool(name="ps_proj", bufs=3, space="PSUM")
                ps_proj = ps_proj_cm.__enter__()
                for which in ("q", "k"):
                    if which == "q":
                        xh_d, xl_d, wh_d, wl_d = xh_q, xl_q, whq, wlq
                        bias_sb, scl, qm_t, qlh_t = bqs_sb, 0.125, qm, qlh
                    else:
                        xh_d, xl_d, wh_d, wl_d = xh_k, xl_k, whk, wlk
                        bias_sb, scl, qm_t, qlh_t = bks_sb, 1.0, km, khl
                    if which == "q":
                        # startup path: split the weight DMAs per pair so
                        # they land on separate queues and finish sooner
                        wh_sb = w_pool.tile([128, NPAIR, ET, 128], BF16, name="wh_in")
                        wl_sb = w_pool.tile([128, NPAIR, ET, 128], BF16, name="wl_in")
                        for p_ in range(NPAIR):
                            nc.sync.dma_start(wh_sb[:, p_], wh_d[:, p_])
                            nc.sync.dma_start(wl_sb[:, p_], wl_d[:, p_])
                    else:
                        wh_sb = w_pool.tile([128, NPAIR, ET, 128], BF16, name="wh_in")
                        nc.sync.dma_start(wh_sb[:], wh_d[:])
                        wl_sb = w_pool.tile([128, NPAIR, ET, 128], BF16, name="wl_in")
                        nc.sync.dma_start(wl_sb[:], wl_d[:])
                    for sblk in range(SBLK):
                        xh_c = stage_pool.tile([128, ET, 512], BF16, name="xh_c")
                        xl_c = stage_pool.tile([128, ET, 512], BF16, name="xl_c")
                        nsp = 8 if (which == "q" and sblk == 0) else 2
                        for eg in range(nsp):
                            w_ = ET // nsp
                            esl = slice(eg * w_, (eg + 1) * w_)
                            nc.sync.dma_start(xh_c[:, esl],
                                              xh_d[sblk, :, esl])
                            nc.sync.dma_start(xl_c[:, esl],
                                              xl_d[sblk, :, esl])
                        if which == "k" and sblk == 1:
                            # prefetch the v-phase inputs while the k DMAs
                            # still have queue headroom, so V matmuls can
                            # start the moment the k matmuls finish
                            load_deferred_consts()
                            wv_hold[0] = w_pool.tile([128, ET, 256], BF16,
                                                     name="wv_in", bufs=1)
                            nc.sync.dma_start(wv_hold[0][:], wv[:])
                            xv0_hold[0] = vstage_pool.tile([128, ET, 512],
                                                           BF16, name="xv0")
                            nc.sync.dma_start(xv0_hold[0][:], xh_v[0])
                        if True:
                            for p in range(NPAIR):
                                psp = ps_proj.tile([128, 512], F32, name="psp")
                                for et in range(ET):
                                    nc.tensor.matmul(psp[:], wh_sb[:, p, et, :],
                                                     xh_c[:, et, :],
                                                     start=(et == 0), stop=False)
                                for et in range(ET):
                                    nc.tensor.matmul(psp[:], wl_sb[:, p, et, :],
                                                     xh_c[:, et, :],
                                                     start=False, stop=False)
                                for et in range(ET):
                                    nc.tensor.matmul(psp[:], wh_sb[:, p, et, :],
                                                     xl_c[:, et, :],
                                                     start=False,
                                                     stop=(et == ET - 1))
                                qex = scr_pool.tile([128, 512], F32, name="qex")
                                nc.scalar.activation(qex[:], psp[:], AF.Identity,
                                                     bias=bias_sb[:, p:p + 1],
                                                     scale=scl)
                                ssl = bass.ts(sblk, 512)
                                for hp in range(2):
                                    h = p * 2 + hp
                                    rsl = slice(hp * 64, hp * 64 + 64)
                                    if which == "q":
                                        # hi target: even -> qm[0:64];
                                        # odd -> qlh[64:128] (both direct)
                                        hi = qm_t[h][0:64, ssl] if hp == 0                                             else qlh_t[h][64:128, ssl]
                                        nc.scalar.activation(
                                            hi, psp[rsl, :], AF.Identity,
                                            bias=bias_sb[rsl, p:p + 1],
                                            scale=scl)
                                        if hp == 0:
                                            nc.vector.scalar_tensor_tensor(
                                                out=qlh_t[h][0:64, ssl],
                                                in0=qex[rsl, :],
                                                scalar=1.0, in1=hi,
                                                op0=ALU.mult, op1=ALU.subtract)
                                            nc.sync.dma_start(
                                                qlh_t[h][64:128, ssl], hi)
                                        else:
                                            ltmp = scr_pool.tile(
                                                [128, 512], BF16, name="ltmp")
                                            nc.vector.scalar_tensor_tensor(
                                                out=ltmp[64:128, :],
                                                in0=qex[rsl, :], scalar=1.0,
                                                in1=hi,
                                                op0=ALU.mult, op1=ALU.subtract)
                                            nc.sync.dma_start(
                                                qm_t[h][0:64, ssl], hi)
                                            nc.sync.dma_start(
                                                qlh_t[h][0:64, ssl],
                                                ltmp[64:128, :])
                                    else:
                                        # k: khl rows 0:64 = hi, 64:128 = lo
                                        if hp == 0:
                                            hi = qm_t[h][0:64, ssl]
                                            nc.scalar.activation(
                                                hi, psp[rsl, :], AF.Identity,
                                                bias=bias_sb[rsl, p:p + 1],
                                                scale=scl)
                                            nc.scalar.activation(
                                                qlh_t[h][0:64, ssl], psp[rsl, :],
                                                AF.Identity,
                                                bias=bias_sb[rsl, p:p + 1],
                                                scale=scl)
                                            ltmp = scr_pool.tile(
                                                [128, 512], BF16, name="ltmp")
                                            nc.vector.scalar_tensor_tensor(
                                                out=ltmp[0:64, :],
                                                in0=qex[rsl, :], scalar=1.0,
                                                in1=hi,
                                                op0=ALU.mult, op1=ALU.subtract)
                                            nc.sync.dma_start(
                                                qlh_t[h][64:128, ssl],
                                                ltmp[0:64, :])
                                        else:
                                            hi = scr_pool.tile(
                                                [128, 512], BF16, name="ktmp")
                                            nc.scalar.activation(
                                                hi[64:128, :], psp[rsl, :],
                                                AF.Identity,
                                                bias=bias_sb[rsl, p:p + 1],
                                                scale=scl)
                                            nc.sync.dma_start(
                                                qm_t[h][0:64, ssl],
                                                hi[64:128, :])
                                            nc.sync.dma_start(
                                                qlh_t[h][0:64, ssl],
                                                hi[64:128, :])
                                            nc.vector.scalar_tensor_tensor(
                                                out=qlh_t[h][64:128, ssl],
                                                in0=qex[rsl, :], scalar=1.0,
                                                in1=hi[64:128, :],
                                                op0=ALU.mult, op1=ALU.subtract)
                ps_proj_cm.__exit__(None, None, None)

                # ---- v projection, interleaved with A jobs + B(h0, ib0) ----
                ps_v_cm = tc.tile_pool(name="ps_v", bufs=3, space="PSUM")
                ps_v = ps_v_cm.__enter__()
                wv_sb = wv_hold[0]
                for sblk in range(SBLK):
                    if sblk == 0:
                        xh_c = xv0_hold[0]
                    else:
                        xh_c = stage_pool.tile([128, ET, 512], BF16, name="xh_c")
                        for eg in range(2):
                            esl = slice(eg * 4, eg * 4 + 4)
                            nc.sync.dma_start(xh_c[:, esl],
                                              xh_v[sblk, :, esl])
                    # V: out [s, d(4 heads)] per 128-s tile, N=256
                    for st2 in range(4):
                        st = sblk * 4 + st2
                        s2 = bass.ts(st2, 128)
                        psv = ps_v.tile([128, 4, 64], F32, name="psv")
                        for et in range(ET):
                            nc.tensor.matmul(psv[:],
                                             xh_c[:, et, s2],
                                             wv_sb[:, et, :],
                                             start=(et == 0),
                                             stop=(et == ET - 1))
                        nc.vector.scalar_tensor_tensor(
                            out=vhh[:, :, st, 0:64],
                            in0=psv[:], scalar=1.0,
                            in1=bvb4[:],
                            op0=ALU.mult, op1=ALU.add)
                        step_all(3)
                ps_v_cm.__exit__(None, None, None)

            # ---- phase 2: attention + out-projection ----
            with (
                tc.tile_pool(name="ctxn", bufs=6) as ctx_pool,
                tc.tile_pool(name="outs", bufs=3) as out_pool,
                tc.tile_pool(name="ps_s", bufs=2, space="PSUM") as ps_s,
                tc.tile_pool(name="ps_o", bufs=2, space="PSUM") as ps_o,
            ):
                def gen_c(ib, p):
                    # C: att @ V-hat, normalize, transpose ctx. The psc
                    # accumulator and pso share the ps_o bank ring.
                    for it in range(4):
                        i0 = ib * 512 + it * 128
                        ctxn = ctx_pool.tile([128, 128], BF16, name="ctxn")
                        for hp in range(2):
                            h = p * 2 + hp
                            psc = ps_o.tile([128, 512], F32, name="pso")
                            for jt in range(JT):
                                nc.tensor.matmul(
                                    psc[:, 0:65],
                                    attT[hp][:, jt, bass.ts(it, 128)],
                                    vh[h][:, jt, :],
                                    start=(jt == 0), stop=(jt == JT - 1))
                            recip = small.tile([128, 1], F32, name="recip")
                            nc.vector.reciprocal(recip[:], psc[:, 64:65])
                            nc.vector.tensor_scalar_mul(
                                ctxn[:, bass.ds(hp * 64, 64)],
                                psc[:, 0:64], recip[:])
                            yield
                        pst = ps_s.tile([128, 128], BF16, name="pst")
                        nc.tensor.transpose(pst[:], ctxn[:], ident_v[:])
                        nc.scalar.copy(ctxT[p][:, bass.ds(i0, 128)], pst[:])

                def gen_o(ib):
                    for it in range(4):
                        i0 = ib * 512 + it * 128
                        for eh in range(2):
                            pso = ps_o.tile([128, 512], F32, name="pso")
                            for ct in range(2):
                                nc.tensor.matmul(pso[:],
                                                 ctxT[ct][:, bass.ds(i0, 128)],
                                                 wo_sb[:, ct, bass.ts(eh, 512)],
                                                 start=(ct == 0), stop=(ct == 1))
                            outsb = out_pool.tile([128, 512], F32, name="outsb")
                            if ib == IB - 1 and it == 3 and eh == 1:
                                # final chunk: DVE copy so it overlaps the
                                # previous chunk's scalar copy at the tail
                                nc.vector.tensor_copy(outsb[:], pso[:])
                            else:
                                nc.scalar.copy(outsb[:], pso[:])
                            nc.sync.dma_start(out_p[bass.ds(i0, 128),
                                                    bass.ts(eh, 512)], outsb[:])
                            yield

                # software pipeline over (ib, head): upcoming heads' A work
                # (vector-paced, from the global deque) is fed between every
                # PE-dense chunk of the current head's B, C and out-proj, so
                # the PE queue never sits on a bare reduce-wait chain and
                # each head's -m rows land ahead of its B.
                heads = [(ib, p, hp) for ib in range(IB)
                         for p in range(NPAIR) for hp in range(2)]
                for ib, p, hp in heads:
                    h = p * 2 + hp
                    drain_until((ib, h))
                    if ib == 0 and h == 0:
                        # B(h0, ib0) may be partially (or fully) consumed
                        # by the v-phase interleave; finish the remainder
                        step_b0(0)
                        if b0_hold[0] != "done":
                            for _ in b0_hold[0]:
                                step_all(1)
                            b0_hold[0] = "done"
                    else:
                        for _ in gen_b(ib, h, hp):
                            step_all(1)
                    if hp == 1:
                        for _ in gen_c(ib, p):
                            step_all(1)
                        if p == 1:
                            for _ in gen_o(ib):
                                step_all(1)
            outer_att.__exit__(None, None, None)
            outer_psb.__exit__(None, None, None)
            outer_psa.__exit__(None, None, None)
            outer_small.__exit__(None, None, None)
    nc.finalize()
    return nc


_NC_CACHE = None


def _get_nc():
    global _NC_CACHE
    if _NC_CACHE is None:
        _NC_CACHE = build_bass()
    return _NC_CACHE


def _prep_core_inputs(inputs, core):
    bf16 = mybir.dt.np(BF16)
    b, hg = core // 4, core % 4
    h0 = hg * HPC
    q, k, v = inputs["q"], inputs["k"], inputs["v"]
    Wq, Wk, Wv = inputs["Wq"], inputs["Wk"], inputs["Wv"]
    bq, bk, bv = inputs["bq"], inputs["bk"], inputs["bv"]
    Wo = inputs["Wo"]

    def split_hl(x):
        xh = x.astype(bf16)
        xl = (x - xh.astype(np.float32)).astype(bf16)
        return xh, xl

    def xt_tiles(x):
        # [S, E] -> [ET, 128, S]
        return np.ascontiguousarray(x.T).reshape(ET, 128, S)

    def chunk_pack(xt):
        # [ET, 128, S] -> [SBLK, 128, ET, 512]: per-partition contiguous
        # 8KB runs per chunk DMA
        return np.ascontiguousarray(
            xt.reshape(ET, 128, SBLK, 512).transpose(2, 1, 0, 3))

    def pack_w(W):
        # [128e, NPAIR, ET, 128]: pair p, e-tile t -> [W[h0+2p] | W[h0+2p+1]]
        out = np.empty((NPAIR, ET, 128, 128), np.float32)
        for p in range(NPAIR):
            pair = np.concatenate([W[h0 + 2 * p], W[h0 + 2 * p + 1]], axis=1)
            out[p] = pair.reshape(ET, 128, 128)
        return np.ascontiguousarray(out.transpose(2, 0, 1, 3))

    def pack_bcol(bias, scale):
        out = np.empty((128, NPAIR), np.float32)
        for p in range(NPAIR):
            out[:, p] = np.concatenate(
                [bias[h0 + 2 * p], bias[h0 + 2 * p + 1]]) * scale
        return out

    xh_q, xl_q = split_hl(xt_tiles(q[b]))
    xh_k, xl_k = split_hl(xt_tiles(k[b]))
    xh_v = xt_tiles(v[b]).astype(bf16)
    whq, wlq = split_hl(pack_w(Wq))
    whk, wlk = split_hl(pack_w(Wk))
    wv_c = np.ascontiguousarray(
        np.concatenate([Wv[h0 + j] for j in range(HPC)],
                       axis=1).reshape(ET, 128, 256).transpose(1, 0, 2)
    ).astype(bf16)

    bvb = np.empty((128, NPAIR, 128), np.float32)
    for p in range(NPAIR):
        bvb[:, p, :] = np.concatenate([bv[h0 + 2 * p], bv[h0 + 2 * p + 1]])[None, :]

    wo_rows = Wo[h0 * DH:(h0 + HPC) * DH, :]  # [256, E]
    wo_c = np.ascontiguousarray(
        wo_rows.reshape(NPAIR, 128, E).transpose(1, 0, 2)).astype(bf16)
    return {
        "xh_q": chunk_pack(xh_q), "xl_q": chunk_pack(xl_q),
        "xh_k": chunk_pack(xh_k), "xl_k": chunk_pack(xl_k),
        "xh_v": chunk_pack(xh_v),
        "whq": whq, "wlq": wlq, "whk": whk, "wlk": wlk, "wv": wv_c,
        "bqs": pack_bcol(bq, 0.125), "bks": pack_bcol(bk, 1.0), "bvb": bvb,
        "wo": wo_c,
    }


def run(inputs, trace=False, **kw):
    inputs = {k: np.asarray(v) for k, v in inputs.items()}
    nc = _get_nc()
    in_maps = [_prep_core_inputs(inputs, c) for c in range(NCORES)]
    res = run_bass_kernel_spmd(nc, in_maps, list(range(NCORES)), trace=trace, **kw)
    bo = inputs["bo"]
    out = np.empty((B, S, E), np.float32)
    for b in range(B):
        acc = res.results[b * 4]["out_p"].astype(np.float32)
        for c in range(b * 4 + 1, b * 4 + 4):
            acc = acc + res.results[c]["out_p"]
        out[b] = acc + bo[None, :]
    return out, res


def kernel(**inputs):
    out, _ = run(inputs)
    return out
